# revision 41
# baseline (speedup 1.0000x reference)
"""Segment-mean (average pooling over sorted segment ids) on 8 TRN2 NeuronCores.

Strategy
--------
segment_ids are sorted, so shard by *segment blocks*: S segments split into
S/128 blocks of 128 segments; each of the 8 cores owns an equal range of
blocks (no cross-core reduction). Each core sorts its 16 blocks by row
count into *slots*; slot j's tile count tau_j = max over the 8 cores of
that slot's block, so padding tracks the sorted order statistics instead
of the global max (~3% fewer tiles). The host unscrambles the slot->block
permutation after the run. The instruction stream is static and shared by
all cores (SPMD).

Features ship in a SINGLE float8e3 (e3m4, 4 mantissa bits) pass with a
trailing ones-column that accumulates counts for free: 1 byte/element.
The PE decodes e3m4 bit-exactly (incl. subnormals, hardware-verified);
measured L2 relative error of the pooled means is 1.34e-2 (gate: 2e-2);
e4m3 would be 2.7e-2 and fail.

Per 128-row tile the device:
  - builds a one-hot oh[i, m] = (windowed_seg_id[row i] == m) in bf16.
    Width-32 windows (the majority) are built 32 tiles at a time in a
    width-major slab [128, 32, G]: is_equal(iota_rep, ids) where iota_rep
    is a physically-tiled constant and ids broadcast with a packed last
    dim — every operand 2-byte/packed/SBUF, so the DVE runs its 2x mode.
    Wider (64/128) windows use per-tile tensor_scalar (4x mode).
  - converts the bf16 one-hot to float8e3 on the otherwise-idle Scalar
    (Activation) engine: LDWEIGHTS streams half the bytes. The PE is
    LDWEIGHTS-bound at this point (~32ns per [64,32] bf16 load).
  - issues 2 matmuls: {rows 0-64, rows 64-128}, each
    psum[w_k : w_k+width] += oh_half.T @ x_half. The halves live on
    distinct PE row-groups with separate PSUM accumulators, so they run
    concurrently and each LDWEIGHTS hides under the other half's matmul.

One-hot windows (w, width) are static per (slot, tile) — with only 8
block instances per slot they are narrow (mostly 32). Tile k=0 uses the
full 128-wide one-hot with start=True to initialize the accumulator.
Padding rows carry id -1 and are zeroed by the one-hot.

Block finalize: sum the two half accumulators, clamp counts to >=1,
reciprocal, multiply, DMA the [128, 128] block mean out (on the idle
Activation HWDGE queue so it never stalls the Sync queue's input chunks).

Host-side input layout is [128 partitions, T tiles, 129B], so every
partition streams long contiguous runs. ids ship bf16 in "section order"
(narrow slab positions first, then w64/w128 for the tensor_scalar path,
converted to f32 once on device) merged with the iota constants into a
single DMA. Input chunks are 6-deep double-buffered: chunk issue+transfer
latency (~2.1us) must hide behind multiple chunk periods of matmul
consumption or the PE stalls and drops out of its 2.4GHz p-state.

Measured on the 2M x 128 / 16K-segment problem: ~161-167 us HW exec
across 8 cores (baseline bf16+fp8 two-pass scheme: 378 us), L2 relative
error 1.341e-2. Per-core floor: ~112 us of e3m4 data on each of 16 DMA
engines, +28 us of PE instruction fetch riding DMA_0, PE ~134 us busy
(LDWEIGHTS-bound: ~17c fixed + ~1.9c/col per load).
"""

import os
import sys
from contextlib import ExitStack

import numpy as np

sys.path.insert(0, "/opt/trn_rl_repo")

import ml_dtypes

from concourse import bass, mybir, tile
from concourse.bass_utils import run_bass_kernel_spmd

BF16 = ml_dtypes.bfloat16
E3M4 = ml_dtypes.float8_e3m4

N_CORES = 8
P = 128      # rows per tile == partitions == matmul contraction dim
D = 128      # feature dim
BLK = 128    # segments per block == psum partitions
WH = D + 1   # rhs width: [feats(128) | ones(1)]
SLAB_G = 32  # tiles per width-32 one-hot slab op

# module-level knobs for test.py
TRACE = False
LAST_EXEC_NS = None
CHUNK = 32   # tiles per input DMA (~528KB each)

_prog_cache = {}


def _ensure_profile_hook():
    """Register the axon NTFF profile hook if the image's antenv lacks it."""
    import types

    try:
        from antenv.axon_hooks import get_axon_ntff_profile_hook  # noqa: F401
        return
    except ImportError:
        pass
    import antenv
    from trn_agent_boot.trn_boot import _ntff_profile_via_ctypes

    mod = types.ModuleType("antenv.axon_hooks")
    _state = {"hook": _ntff_profile_via_ctypes("/opt/axon/libaxon_pjrt.so")}
    mod.set_axon_ntff_profile_hook = lambda h: _state.__setitem__("hook", h)
    mod.get_axon_ntff_profile_hook = lambda: _state["hook"]
    sys.modules["antenv.axon_hooks"] = mod
    antenv.axon_hooks = mod


def _split_excess_waits(nc, cap=1):
    """Walrus enforces a limit of one sync-wait command per instruction.
    Tile can emit more. Split the excess into wait-only NOPs placed
    immediately before the instruction on the same engine."""
    ctr = [0]
    for f in nc.m.functions:
        for blk in f.blocks:
            insts = blk.instructions
            out = []
            changed = False
            for inst in insts:
                si = inst.sync_info
                waits = list(si.on_wait) if si is not None and si.on_wait else []
                if len(waits) > cap:
                    excess, keep = waits[:-cap], waits[-cap:]
                    for i in range(0, len(excess), cap):
                        chunk = excess[i : i + cap]
                        ctr[0] += 1
                        nop = mybir.InstNoOp(
                            name=f"W-split-{ctr[0]}",
                            engine=inst.engine,
                            sync_info=mybir.SyncInfo(on_wait=chunk, on_update=[]),
                            ins=[],
                            outs=[],
                            bass_nofuse=True,
                        )
                        out.append(nop)
                    inst.sync_info = mybir.SyncInfo(
                        on_wait=keep, on_update=list(si.on_update) if si.on_update else []
                    )
                    changed = True
                out.append(inst)
            if changed:
                blk.instructions = out
    return nc


def _sections(plans):
    """Assign every tile instance (slot j, tile k) an ids-buffer column,
    grouped by one-hot width: w32 instances first (slab-built, bf16),
    then w64, then w128 (tensor_scalar path, f32).
    Returns (col_of dict, n32p, npos)."""
    w32, w64, w128 = [], [], []
    for j, plan in enumerate(plans):
        for k, (_, width) in enumerate(plan):
            (w32 if width <= 32 else w64 if width == 64 else w128).append((j, k))
    n32p = -(-len(w32) // SLAB_G) * SLAB_G
    col_of = {}
    for i, jk in enumerate(w32):
        col_of[jk] = i
    for i, jk in enumerate(w64):
        col_of[jk] = n32p + i
    for i, jk in enumerate(w128):
        col_of[jk] = n32p + len(w64) + i
    npos = n32p + len(w64) + len(w128)
    return col_of, n32p, npos


def _build_program(taus: tuple, plans: tuple):
    """One SPMD Bass program: len(taus) slots, tau_j tiles each."""
    nc = bass.Bass()
    nslots = len(taus)
    offs = np.concatenate([[0], np.cumsum(taus)])
    T = int(offs[-1])
    col_of, n32p, npos = _sections(plans)
    nwide = npos - n32p

    # one merged const param: [iota_rep(1024) | iota_lin(128) | ids32 | idsw(bf16)]
    NIOTA = SLAB_G * 32 + BLK
    NCONST = NIOTA + n32p + nwide + 4
    xh = nc.declare_dram_parameter("xh", [P, T, WH], mybir.dt.float8e3, isOutput=False)
    cst = nc.declare_dram_parameter("cst", [P, NCONST], mybir.dt.bfloat16, isOutput=False)
    out = nc.declare_dram_parameter("out", [nslots, BLK, D], mybir.dt.float32, isOutput=True)

    with tile.TileContext(nc) as tc, ExitStack() as ctx:
        # wide one-hots and slabs are prefetched one slot ahead (the prepass
        # below), so pools must hold two slots' worth
        wide_of_slot = [
            [(k, col_of[(j, k)], plans[j][k]) for k in range(taus[j]) if plans[j][k][1] > 32]
            for j in range(nslots)
        ]
        maxwide = max(len(w) for w in wide_of_slot)
        slab_first_slot = {}
        for j in range(nslots):
            for k in range(taus[j]):
                if plans[j][k][1] <= 32:
                    s = col_of[(j, k)] // SLAB_G
                    slab_first_slot.setdefault(s, j)
        slabs_of_slot = [
            [s for s, jj in slab_first_slot.items() if jj == j] for j in range(nslots)
        ]
        maxslab = max(len(s) for s in slabs_of_slot)

        const = ctx.enter_context(tc.tile_pool(name="const", bufs=1))
        # deep input pipeline: chunk issue+transfer latency must hide behind
        # multiple chunk periods of matmul consumption, or the PE stalls and
        # drops out of its max p-state
        xp = ctx.enter_context(tc.tile_pool(name="xp", bufs=6))
        slabp = ctx.enter_context(tc.tile_pool(name="slabp", bufs=2 * maxslab + 2))
        ohp = ctx.enter_context(tc.tile_pool(name="ohp", bufs=2 * maxwide + 2))
        psp = ctx.enter_context(tc.tile_pool(name="psp", bufs=4, space="PSUM"))
        finp = ctx.enter_context(tc.tile_pool(name="finp", bufs=2))

        cst_sb = const.tile([P, NCONST], mybir.dt.bfloat16)
        nc.sync.dma_start(cst_sb[:], cst[:])
        ids_sb = cst_sb[:, NIOTA : NIOTA + n32p]
        # wide-path is_equal needs an f32 scalar operand: convert once on DVE
        idsw_sb = const.tile([P, nwide + 4], mybir.dt.float32)
        nc.vector.tensor_copy(idsw_sb[:], cst_sb[:, NIOTA + n32p :])

        iota_rep = cst_sb[:, 0 : SLAB_G * 32].rearrange("p (j g) -> p j g", g=SLAB_G)
        iota_lin = cst_sb[:, SLAB_G * 32 : NIOTA]

        slabs = {}

        def get_slab(s):
            if s not in slabs:
                oh = slabp.tile([P, 32, SLAB_G], mybir.dt.bfloat16, tag="slab")
                c0 = s * SLAB_G
                nc.vector.tensor_tensor(
                    oh[:],
                    iota_rep,
                    ids_sb[:, c0 : c0 + SLAB_G]
                    .rearrange("p (o g) -> p o g", o=1)
                    .broadcast_to((P, 32, SLAB_G)),
                    mybir.AluOpType.is_equal,
                )
                slabs[s] = oh
            return slabs[s]

        ohw_cache = {}

        def prepass(j):
            """Emit slot j's one-hot ops ahead of the previous slot's
            finalize, so the PE never waits on the DVE at slot entry."""
            for s in sorted(slabs_of_slot[j]):
                get_slab(s)
            for k, col, (wbase, width) in wide_of_slot[j]:
                ohw = ohp.tile([P, BLK], mybir.dt.bfloat16, tag="ohw")
                wc = col - n32p
                nc.vector.tensor_scalar(
                    ohw[:, :width],
                    iota_lin[:, :width],
                    idsw_sb[:, wc : wc + 1],
                    None,
                    mybir.AluOpType.is_equal,
                )
                ohw_cache[(j, k)] = ohw

        prepass(0)
        for j in range(nslots):
            tau = taus[j]
            plan = plans[j]
            # two K=64 row-half accumulators on distinct PE row-groups
            ps_a = psp.tile([P, WH], mybir.dt.float32, tag="psA")
            ps_b = psp.tile([P, WH], mybir.dt.float32, tag="psB")
            for k0 in range(0, tau, CHUNK):
                g = min(CHUNK, tau - k0)
                t0 = int(offs[j]) + k0
                ch = xp.tile([P, CHUNK, WH], mybir.dt.float8e3, tag="xh")
                nc.sync.dma_start(ch[:, :g, :], xh[:, t0 : t0 + g, :])
                for kk in range(g):
                    k = k0 + kk
                    wbase, width = plan[k]
                    col = col_of[(j, k)]
                    if width <= 32:
                        # narrow (8/16/32) windows all read the w32 slab; a
                        # narrower lhsT slice just loads fewer PE columns
                        slab = get_slab(col // SLAB_G)
                        lhs = slab[:, :width, col % SLAB_G]
                    else:
                        lhs = ohw_cache.pop((j, k))[:, :width]
                    nc.tensor.matmul(
                        ps_a[wbase : wbase + width, :],
                        lhs[0:64, :],
                        ch[0:64, kk, :],
                        tile_position=(0, wbase),
                        start=(k == 0),
                        stop=(k == tau - 1),
                        skip_group_check=True,
                    )
                    nc.tensor.matmul(
                        ps_b[wbase : wbase + width, :],
                        lhs[64:128, :],
                        ch[64:128, kk, :],
                        tile_position=(64, wbase),
                        start=(k == 0),
                        stop=(k == tau - 1),
                        skip_group_check=True,
                    )
            if j + 1 < nslots:
                prepass(j + 1)
            # finalize slot: mean = (half_a + half_b) / max(count, 1)
            # (one operand must bounce through SBUF: walrus allows a single
            # PSUM input per DVE op)
            sums = finp.tile([P, WH], mybir.dt.float32, tag="sums")
            nc.vector.tensor_copy(sums[:], ps_a[:])
            nc.vector.tensor_add(sums[:], sums[:], ps_b[:])
            cnt = finp.tile([P, 1], mybir.dt.float32, tag="cnt")
            nc.vector.tensor_scalar_max(cnt[:], sums[:, D : D + 1], 1.0)
            rcp = finp.tile([P, 1], mybir.dt.float32, tag="rcp")
            nc.vector.reciprocal(rcp[:], cnt[:])
            osb = finp.tile([P, D], mybir.dt.float32, tag="osb")
            nc.vector.tensor_scalar(
                osb[:], sums[:, 0:D], rcp[:], None, mybir.AluOpType.mult
            )
            # output DMA rides the (idle) Activation HWDGE queue: its wait on
            # the finalize chain must not stall the Sync queue, which is busy
            # issuing the next slot's input chunks
            nc.scalar.dma_start(out[j], osb[:])
    return _split_excess_waits(nc)


def _plan_windows_slot(segment_ids, bounds, blocks, tau):
    """Window (base w, width) per tile k for one slot, valid for each of the
    slot's block instances (one per core). Width-32 windows start at
    multiples of 32, width-64 at {0, 64}, width-128 at 0. Tile 0 always
    gets (0, 128) — it initializes the whole accumulator."""
    lo = np.full(tau, BLK, dtype=np.int64)
    hi = np.full(tau, -1, dtype=np.int64)
    for gb in blocks:
        r0, r1 = int(bounds[gb]), int(bounds[gb + 1])
        n = r1 - r0
        if n == 0:
            continue
        sid = segment_ids[r0:r1]
        base = gb * BLK
        kmax = -(-n // P)
        for k in range(kmax):
            a = sid[k * P] - base
            bnd = sid[min((k + 1) * P, n) - 1] - base
            if a < lo[k]:
                lo[k] = a
            if bnd > hi[k]:
                hi[k] = bnd
    plan = []
    for k in range(tau):
        if k == 0 or hi[k] < 0:
            plan.append((0, BLK))
            continue
        chosen = None
        # psum output base must be 32-aligned; widths 8/16 ride the w32 slab
        # one-hot but load proportionally fewer PE columns (LDWEIGHTS is
        # ~17c fixed + ~1.9c/col, and the PE is LDW-bound)
        for width, step in ((8, 32), (16, 32), (32, 32), (64, 64), (128, 128)):
            for w in range(0, BLK - width + 1, step):
                if w <= lo[k] and hi[k] < w + width:
                    chosen = (w, width)
                    break
            if chosen:
                break
        assert chosen is not None  # width=128, w=0 always covers
        plan.append(chosen)
    return tuple(plan)


def kernel(feats, segment_ids, num_segments):
    global LAST_EXEC_NS
    feats = np.asarray(feats, dtype=np.float32)
    segment_ids = np.asarray(segment_ids, dtype=np.int32)
    S = int(num_segments)
    N = feats.shape[0]
    assert feats.shape[1] == D
    assert S % (N_CORES * BLK) == 0, f"num_segments={S} must divide into 8x128 blocks"
    seg_per_core = S // N_CORES
    nblk = seg_per_core // BLK

    # rows of each 128-segment block (ids are sorted)
    bounds = np.searchsorted(segment_ids, np.arange(0, S + 1, BLK))
    rows_per_block = np.diff(bounds)

    # slot assignment: each core sorts its blocks by descending row count;
    # slot j's tau is the max over cores of slot j's block
    perm = np.empty((N_CORES, nblk), dtype=np.int64)
    for c in range(N_CORES):
        blocks = np.arange(c * nblk, (c + 1) * nblk)
        perm[c] = blocks[np.argsort(-rows_per_block[blocks], kind="stable")]
    taus = tuple(
        max(
            1,
            int(-(-int(max(rows_per_block[perm[c][j]] for c in range(N_CORES))) // P)),
        )
        for j in range(nblk)
    )
    offs = np.concatenate([[0], np.cumsum(taus)])
    T = int(offs[-1])

    plans = tuple(
        _plan_windows_slot(
            segment_ids, bounds, [int(perm[c][j]) for c in range(N_CORES)], taus[j]
        )
        for j in range(nblk)
    )
    col_of, n32p, npos = _sections(plans)
    nwide = npos - n32p

    iota_np = np.ascontiguousarray(
        np.broadcast_to(
            np.concatenate(
                [
                    np.repeat(np.arange(32, dtype=np.float32), SLAB_G),
                    np.arange(BLK, dtype=np.float32),
                ]
            ),
            (P, SLAB_G * 32 + BLK),
        )
    ).astype(BF16)

    in_maps = []
    for c in range(N_CORES):
        idx = np.zeros((T, P), dtype=np.int64)
        sid = np.full((T, P), -1.0, dtype=np.float32)
        for j in range(nblk):
            gb = int(perm[c][j])
            r0, r1 = int(bounds[gb]), int(bounds[gb + 1])
            n = r1 - r0
            tau = taus[j]
            assert n <= tau * P
            o = int(offs[j])
            flat_idx = idx[o : o + tau].reshape(-1)
            flat_sid = sid[o : o + tau].reshape(-1)
            flat_idx[:n] = np.arange(r0, r1)
            local = segment_ids[r0:r1].astype(np.float32) - gb * BLK
            koff = np.repeat(
                np.asarray([plans[j][k][0] for k in range(tau)], dtype=np.float32), P
            )[:n]
            flat_sid[:n] = local - koff
        idxT = idx.T  # [P, T]
        f = feats[idxT.reshape(-1)]  # [P*T, D]; pad rows point at row 0, masked
        Xc = np.empty((P, T, WH), dtype=E3M4)
        Xc[:, :, 0:D] = f.astype(E3M4).reshape(P, T, D)
        Xc[:, :, D] = 1.0
        # ids in section order (w32 slab columns, then wide columns); all
        # values are small ints, bf16-exact. Shipped merged with the iota
        # constants as one DMA.
        idsall = np.full((P, npos + 4), -1.0, dtype=np.float32)
        for j in range(nblk):
            for k in range(taus[j]):
                idsall[:, col_of[(j, k)]] = sid[int(offs[j]) + k]
        cstbuf = np.concatenate([iota_np, idsall.astype(BF16)], axis=1)
        in_maps.append({"xh": Xc, "cst": np.ascontiguousarray(cstbuf)})

    key = (taus, plans)
    if key not in _prog_cache:
        _prog_cache[key] = _build_program(taus, plans)
    nc = _prog_cache[key]

    if TRACE:
        _ensure_profile_hook()
    # the very first execution of a freshly compiled NEFF occasionally hits a
    # transient NRT_EXEC_UNIT_UNRECOVERABLE; retry a couple of times
    last_exc = None
    for attempt in range(3):
        try:
            res = run_bass_kernel_spmd(
                nc, in_maps, core_ids=list(range(N_CORES)), trace=TRACE
            )
            break
        except Exception as e:  # noqa: BLE001
            last_exc = e
            import time as _time

            _time.sleep(2.0)
    else:
        raise last_exc
    LAST_EXEC_NS = res.exec_time_ns
    result = np.empty((S, D), dtype=np.float32)
    for c in range(N_CORES):
        o = np.asarray(res.results[c]["out"])  # [nblk, BLK, D]
        for j in range(nblk):
            gb = int(perm[c][j])
            result[gb * BLK : (gb + 1) * BLK] = o[j]
    return result
